# revision 1
# baseline (speedup 1.0000x reference)
"""Trainium2 Bass kernel for nn_AttentionHead (B=4, S=2048, D=1024, d_qk=d_vo=128).

Sharding: 8 cores = 4 batches x 2 interleaved query-tile sets.
Core c handles batch b=c//2 and query tiles {j, j+2, ..., j+14} (j=c%2).
Keys/values are recomputed per core (no collectives).

Per-core dataflow (all matmuls bf16 with fp32 PSUM accumulation):
  - host pre-transposes/permutes enc to encT [D, S] bf16, owned q rows first
  - q^T, k^T, v^T projections via W as stationary operand
  - v^T -> v natural via 128x128 SBUF->SBUF DMA (XBAR) transposes, with a
    ones column appended so one matmul yields both att@v and softmax-Z
  - scores computed transposed (S^T[sk, sq]) so softmax needs no transposes;
    logits are tiny (|x| < 3), so exp is applied without max-subtraction
  - causal masking: multiply exp tiles by a triangular / per-core mask tile
  - out-projection with avT as stationary; 1/Z folded into the PSUM->SBUF copy
"""

import os
import sys

import numpy as np

for _p in ("/opt/trn_rl_repo", os.path.expanduser("~/.axon_site/_ro/trn_rl_repo")):
    if os.path.isdir(_p) and _p not in sys.path:
        sys.path.insert(0, _p)

import ml_dtypes

import concourse.bass as bass
import concourse.mybir as mybir
import concourse.tile as tile
from concourse.bass import ts
from concourse.masks import make_identity

B, S, D, E = 4, 2048, 1024, 128
P = 128
NT = S // P          # 16 key tiles
NQT = 8              # owned query tiles per core
BF16 = mybir.dt.bfloat16
F32 = mybir.dt.float32
SCALE = 1.0 / float(np.sqrt(E))

LAST_RESULTS = None  # BassKernelResults of the most recent run (for test harness)


def _emit(tc, encT_d, wq_d, wk_d, wv_d, wo_d, masks_d, out_d, opts=None):
    O = dict(proj_paired=False, interleave=True, outproj_in_chunk=False,
             work_bufs=6, outp_bufs=3, enc_split=True, reps=1, loop_reps=0,
             body="full", psum_s_bufs=2, psum_kv_bufs=2, dve_transpose=False,
             enc_alt_eng=False)
    if opts:
        O.update(opts)
    nc = tc.nc
    from contextlib import ExitStack

    with ExitStack() as ctx:
        const = ctx.enter_context(tc.tile_pool(name="const", bufs=1))
        encp = ctx.enter_context(tc.tile_pool(name="encp", bufs=8))
        proj = ctx.enter_context(tc.tile_pool(name="proj", bufs=1))
        work = ctx.enter_context(tc.tile_pool(name="work", bufs=O["work_bufs"]))
        outp = ctx.enter_context(tc.tile_pool(name="outp", bufs=O["outp_bufs"]))
        psum_s = ctx.enter_context(tc.tile_pool(name="psum_s", bufs=O["psum_s_bufs"], space="PSUM"))
        psum_av = ctx.enter_context(tc.tile_pool(name="psum_av", bufs=4, space="PSUM"))
        psum_kv = ctx.enter_context(tc.tile_pool(name="psum_kv", bufs=O["psum_kv_bufs"], space="PSUM"))

        # constants
        ident = const.tile([P, P], BF16, tag="ident")
        make_identity(nc, ident)
        masks_sb = const.tile([P, 2, P], BF16, tag="masks")
        nc.sync.dma_start(masks_sb[:, 0, :], masks_d[0])
        nc.sync.dma_start(masks_sb[:, 1, :], masks_d[1])

        # weights
        wq_sb = const.tile([P, 8, E], BF16, tag="wq")
        wk_sb = const.tile([P, 8, E], BF16, tag="wk")
        wv_sb = const.tile([P, 8, E], BF16, tag="wv")
        wo_sb = const.tile([P, D], BF16, tag="wo")
        nc.sync.dma_start(wq_sb[:], wq_d.rearrange("(o p) e -> p o e", p=P))
        nc.sync.dma_start(wk_sb[:], wk_d.rearrange("(o p) e -> p o e", p=P))
        nc.sync.dma_start(wv_sb[:], wv_d.rearrange("(o p) e -> p o e", p=P))
        nc.sync.dma_start(wo_sb[:], wo_d[:])

        # encT resident in SBUF: 8 tiles [128, 2048] bf16
        enc_sb = [encp.tile([P, S], BF16, tag="enc", name=f"enc_{i}") for i in range(8)]

        def load_enc():
            if O["body"] == "dmaout":
                nc.sync.dma_start(enc_sb[0][:, 0:512], encT_d[0:P, 0:512])
                return
            if O["body"] == "dmasplit":
                i = 0
                for oo in range(8):
                    for c in range(4):
                        eng = nc.sync if i % 2 == 0 else nc.scalar
                        eng.dma_start(
                            enc_sb[oo][:, ts(c, 512)], encT_d[ts(oo, P), ts(c, 512)]
                        )
                        i += 1
                return
            if O["enc_split"]:
                i = 0
                for c in (0, 2, 1, 3):
                    for oo in range(8):
                        eng = nc.scalar if (O["enc_alt_eng"] and i % 2) else nc.sync
                        eng.dma_start(
                            enc_sb[oo][:, ts(c, 512)], encT_d[ts(oo, P), ts(c, 512)]
                        )
                        i += 1
            else:
                for oo in range(8):
                    nc.sync.dma_start(enc_sb[oo][:], encT_d[ts(oo, P), :])

        if False:
            pass

        # ---- projections ----
        qT_sb = proj.tile([P, NQT * P], BF16, tag="qT")      # [e, sq]  (1024 owned q)
        kT_sb = proj.tile([P, S], BF16, tag="kT")            # [e, sk]
        vT_sb = proj.tile([P, S], BF16, tag="vT")            # [e, sk]
        v_sb = proj.tile([P, NT, E + 1], BF16, tag="v")      # [sk, t, e|1]
        nc.gpsimd.memset(v_sb[:, :, E : E + 1], 1.0)

        def project(w_sb, dst_sb, chunks, name, rep=0):
            if O["proj_paired"]:
                ps = [
                    psum_kv.tile([P, 512], F32, tag="pkv", name=f"pj_{rep}_{name}_{c}")
                    for c in chunks
                ]
                for oo in range(8):
                    for i, c in enumerate(chunks):
                        nc.tensor.matmul(
                            ps[i],
                            w_sb[:, oo, :],
                            enc_sb[oo][:, ts(c, 512)],
                            start=(oo == 0),
                            stop=(oo == 7),
                        )
                for i, c in enumerate(chunks):
                    nc.vector.tensor_copy(dst_sb[:, ts(c, 512)], ps[i])
            else:
                for c in chunks:
                    ps = psum_kv.tile([P, 512], F32, tag="pkv", name=f"pj_{rep}_{name}_{c}")
                    for oo in range(8):
                        nc.tensor.matmul(
                            ps,
                            w_sb[:, oo, :],
                            enc_sb[oo][:, ts(c, 512)],
                            start=(oo == 0),
                            stop=(oo == 7),
                        )
                    nc.vector.tensor_copy(dst_sb[:, ts(c, 512)], ps)

        def v_natural(kc, rep=0):
            # v^T 512-col chunk kc -> natural-layout tiles
            for t in range(kc * 4, kc * 4 + 4):
                if O["dve_transpose"]:
                    nc.vector.transpose(v_sb[:, t, 0:E], vT_sb[:, ts(t, P)])
                    continue
                tp = psum_s.tile([P, 512], F32, tag="sc", name=f"vtp_{rep}_{t}")
                tpb = tp.bitcast(BF16)
                nc.tensor.transpose(tpb[:, :P], vT_sb[:, ts(t, P)], ident)
                nc.scalar.copy(v_sb[:, t, 0:E], tpb[:, :P])

        # ---- attention (transposed scores), per 512-wide query chunk ----
        rz_sb = proj.tile([P, NQT], F32, tag="rz")           # 1/Z per q row
        avT_sb = proj.tile([P, NQT, P], BF16, tag="avT")     # [e, t, sq]

        def attention(chunk, s_list, rep=0):
            tlo, thi = chunk * 4, chunk * 4 + 4
            av_ps = [
                psum_av.tile([P, E + 1], F32, tag="av", name=f"av_{rep}_{chunk}_{i}")
                for i in range(4)
            ]
            started = [False] * 4
            for s in s_list:
                base = s if s < 8 else s - 8
                first_t = max(base, tlo)
                if first_t >= thi:
                    continue
                W = (thi - first_t) * P
                col0 = first_t * P

                sc = psum_s.tile([P, 512], F32, tag="sc")
                nc.tensor.matmul(
                    sc[:, :W],
                    kT_sb[:, ts(s, P)],
                    qT_sb[:, col0 : col0 + W],
                    start=True,
                    stop=True,
                )
                ex = work.tile([P, 512], BF16, tag="ex")
                nc.scalar.activation(
                    ex[:, :W], sc[:, :W], mybir.ActivationFunctionType.Exp, scale=SCALE
                )
                if tlo <= base < thi:
                    # boundary tile: triangular (s<8) or per-core (s>=8) mask
                    m = 0 if s < 8 else 1
                    nc.vector.tensor_mul(ex[:, 0:P], ex[:, 0:P], masks_sb[:, m, :])
                for i, t in enumerate(range(first_t, thi)):
                    nc.tensor.matmul(
                        av_ps[t - tlo],
                        ex[:, ts(i, P)],
                        v_sb[:, s, :],
                        start=not started[t - tlo],
                        stop=(s == t + 8),
                    )
                    started[t - tlo] = True

            # finalize + output projection for this chunk
            for t in range(tlo, thi):
                ps = av_ps[t - tlo]
                nc.vector.reciprocal(rz_sb[:, t : t + 1], ps[:, E : E + 1])
                avn = work.tile([P, P], BF16, tag="avn")
                nc.scalar.copy(avn, ps[:, 0:E])
                if O["dve_transpose"]:
                    nc.vector.transpose(avT_sb[:, t, :], avn)
                    continue
                tp = psum_s.tile([P, 512], F32, tag="sc")
                tpb = tp.bitcast(BF16)
                nc.tensor.transpose(tpb[:, :P], avn, ident)
                nc.scalar.copy(avT_sb[:, t, :], tpb[:, :P])
            if not O["outproj_in_chunk"]:
                return
            for t in range(tlo, thi):
                ob = outp.tile([P, D], F32, tag="ob")
                for dc in range(2):
                    po = psum_kv.tile([P, 512], F32, tag="pkv")
                    nc.tensor.matmul(
                        po, avT_sb[:, t, :], wo_sb[:, ts(dc, 512)],
                        start=True, stop=True,
                    )
                    nc.vector.tensor_scalar_mul(
                        ob[:, ts(dc, 512)], po, rz_sb[:, t : t + 1]
                    )
                nc.sync.dma_start(out_d[ts(t, P), :], ob)

        from contextlib import nullcontext

        def one_rep(rep):
            load_enc()
            if O["body"] == "dma":
                for t in range(NQT):
                    ob = outp.tile([P, D], BF16, tag="ob", name=f"obd_{rep}_{t}")
                    nc.vector.tensor_copy(ob[:, 0:256], enc_sb[t][:, 0:512].bitcast(F32))
                    nc.sync.dma_start(out_d[ts(t, P), :], ob)
                return
            if O["body"] == "dmain":
                ob = outp.tile([P, D], BF16, tag="ob", name=f"obi_{rep}")
                nc.vector.tensor_copy(ob[:, 0:512], enc_sb[0][:, 0:512])
                nc.sync.dma_start(out_d[0:P, :], ob)
                return
            if O["body"] == "dmaout":
                for t in range(NQT):
                    ob = outp.tile([P, D], BF16, tag="ob", name=f"obo_{rep}_{t}")
                    nc.vector.tensor_copy(ob[:, 0:512], enc_sb[0][:, 0:512])
                    nc.sync.dma_start(out_d[ts(t, P), :], ob)
                return
            if O["body"] == "dmasplit":
                for t in range(NQT):
                    ob = outp.tile([P, D], BF16, tag="ob", name=f"obs_{rep}_{t}")
                    nc.vector.tensor_copy(ob[:, 0:256], enc_sb[t][:, 0:512].bitcast(F32))
                    for h in range(2):
                        eng = nc.sync if h == 0 else nc.scalar
                        eng.dma_start(out_d[ts(t, P), ts(h, 512)], ob[:, ts(h, 512)])
                return
            if O["body"] == "proj":
                for nm, w, dst in (("q", wq_sb, qT_sb), ("k", wk_sb, kT_sb), ("v", wv_sb, vT_sb)):
                    cs = [0, 1] if nm == "q" else [0, 1, 2, 3]
                    for c in cs:
                        project(w, dst, [c], f"{nm}{c}", rep)
                for kc in range(4):
                    v_natural(kc, rep)
                for t in range(NQT):
                    ob = outp.tile([P, D], F32, tag="ob", name=f"obp_{rep}_{t}")
                    nc.vector.tensor_copy(ob[:, 0:512], kT_sb[:, 0:512])
                    nc.vector.tensor_copy(ob[:, 512:1024], vT_sb[:, 0:512])
                    nc.sync.dma_start(out_d[ts(t, P), :], ob)
                return
            if O["interleave"]:
                project(wq_sb, qT_sb, [0, 1], "q", rep)
                project(wk_sb, kT_sb, [0, 2], "k02", rep)
                project(wv_sb, vT_sb, [0, 2], "v02", rep)
                v_natural(0, rep)
                v_natural(2, rep)
                attention(0, [0, 1, 2, 3, 8, 9, 10, 11], rep)
                project(wk_sb, kT_sb, [1, 3], "k13", rep)
                project(wv_sb, vT_sb, [1, 3], "v13", rep)
                v_natural(1, rep)
                v_natural(3, rep)
                attention(1, list(range(16)), rep)
            else:
                project(wk_sb, kT_sb, [0, 2], "k02", rep)
                project(wk_sb, kT_sb, [1, 3], "k13", rep)
                project(wq_sb, qT_sb, [0, 1], "q", rep)
                project(wv_sb, vT_sb, [0, 2], "v02", rep)
                project(wv_sb, vT_sb, [1, 3], "v13", rep)
                for kc in range(4):
                    v_natural(kc, rep)
                attention(0, [0, 1, 2, 3, 8, 9, 10, 11], rep)
                attention(1, list(range(16)), rep)
            if O["outproj_in_chunk"]:
                return
            for t in range(NQT):
                ob = outp.tile([P, D], BF16, tag="ob", name=f"ob_{rep}_{t}")
                for dc in range(2):
                    po = psum_kv.tile([P, 512], F32, tag="pkv")
                    nc.tensor.matmul(
                        po, avT_sb[:, t, :], wo_sb[:, ts(dc, 512)],
                        start=True, stop=True,
                    )
                    nc.vector.tensor_scalar_mul(
                        ob[:, ts(dc, 512)], po, rz_sb[:, t : t + 1]
                    )
                nc.sync.dma_start(out_d[ts(t, P), :], ob)

        if O["loop_reps"]:
            with tc.For_i(0, O["loop_reps"], 1):
                one_rep(0)
        else:
            for rep in range(O["reps"]):
                one_rep(rep)


def _split_multiwaits(nc):
    """This walrus build rejects instructions carrying more than one semaphore
    wait ("Too many sync wait commands"). Split extras onto standalone
    InstEventSemaphore carriers on the same engine, inserted just before, which
    preserves per-engine ordering and therefore the same gating semantics."""
    n = 0
    for f in nc.m.functions:
        for blk in f.blocks:
            out = []
            changed = False
            for inst in blk.instructions:
                si = inst.sync_info
                if si is not None and si.on_wait and len(si.on_wait) > 1:
                    waits = list(si.on_wait)
                    for i, w in enumerate(waits[:-1]):
                        ev = mybir.InstEventSemaphore(
                            name=f"{inst.name}_xw{i}", ins=[], outs=[]
                        )
                        ev.engine = inst.engine
                        ev.sync_info = mybir.SyncInfo(on_wait=[w], on_update=[])
                        out.append(ev)
                        n += 1
                    inst.sync_info = mybir.SyncInfo(
                        on_wait=[waits[-1]], on_update=list(si.on_update)
                    )
                    changed = True
                out.append(inst)
            if changed:
                blk.instructions = out
    return n


def build_nc(split=True, opts=None):
    nc = bass.Bass("TRN2")
    encT = nc.dram_tensor("encT", [D, S], BF16, kind="ExternalInput")
    wq = nc.dram_tensor("wq", [D, E], BF16, kind="ExternalInput")
    wk = nc.dram_tensor("wk", [D, E], BF16, kind="ExternalInput")
    wv = nc.dram_tensor("wv", [D, E], BF16, kind="ExternalInput")
    wo = nc.dram_tensor("wo", [E, D], BF16, kind="ExternalInput")
    masks = nc.dram_tensor("masks", [2, P, P], BF16, kind="ExternalInput")
    out = nc.dram_tensor("out", [NQT * P, D], BF16, kind="ExternalOutput")
    with tile.TileContext(nc) as tc:
        _emit(tc, encT[:], wq[:], wk[:], wv[:], wo[:], masks[:], out[:], opts)
    if split:
        _split_multiwaits(nc)
    return nc


_NC = None


def _get_nc():
    global _NC
    if _NC is None:
        _NC = build_nc()
    return _NC


def _perm_rows(j):
    tiles = [2 * p + j for p in range(8)] + [2 * m + 1 - j for m in range(8)]
    return np.concatenate([np.arange(t * P, (t + 1) * P) for t in tiles])


def make_in_maps(encodings, W_q, W_k, W_v, W_o):
    bf = ml_dtypes.bfloat16
    enc16 = np.asarray(encodings).astype(bf)
    wq16 = np.ascontiguousarray(np.asarray(W_q).astype(bf))
    wk16 = np.ascontiguousarray(np.asarray(W_k).astype(bf))
    wv16 = np.ascontiguousarray(np.asarray(W_v).astype(bf))
    wo16 = np.ascontiguousarray(np.asarray(W_o).astype(bf))
    tri = (np.arange(P)[:, None] <= np.arange(P)[None, :]).astype(bf)
    in_maps = []
    for core in range(8):
        b, j = core // 2, core % 2
        rows = _perm_rows(j)
        encT = np.ascontiguousarray(enc16[b].T[:, rows])
        pmask = np.full((P, P), float(j), dtype=bf)
        masks = np.ascontiguousarray(np.stack([tri, pmask]))
        in_maps.append(
            {"encT": encT, "wq": wq16, "wk": wk16, "wv": wv16, "wo": wo16,
             "masks": masks}
        )
    return in_maps


def _is_causal(mask):
    m = np.asarray(mask)
    causal = np.triu(np.ones((S, S), dtype=bool), k=1)
    return all(np.array_equal(m[b], causal) for b in range(B))


def _numpy_fallback(encodings, mask, W_q, W_k, W_v, W_o):
    enc = np.asarray(encodings, np.float32)
    out = np.empty((B, S, D), np.float32)
    for b in range(B):
        q = enc[b] @ W_q
        k = enc[b] @ W_k
        v = enc[b] @ W_v
        sims = (q @ k.T) / np.float32(np.sqrt(E))
        sims = np.where(np.asarray(mask[b]), np.float32(-1e9), sims)
        sims -= sims.max(-1, keepdims=True)
        e = np.exp(sims)
        attn = e / e.sum(-1, keepdims=True)
        out[b] = (attn @ v) @ W_o
    return out


def kernel(encodings, mask, W_q, W_k, W_v, W_o):
    global LAST_RESULTS
    if not _is_causal(mask):
        return _numpy_fallback(encodings, mask, W_q, W_k, W_v, W_o)

    from concourse import bass_utils

    nc = _get_nc()
    in_maps = make_in_maps(encodings, W_q, W_k, W_v, W_o)
    trace = os.environ.get("KERNEL_TRACE", "0") == "1"
    try:
        res = bass_utils.run_bass_kernel_spmd(
            nc, in_maps, core_ids=list(range(8)), trace=trace
        )
    except ModuleNotFoundError:
        res = bass_utils.run_bass_kernel_spmd(
            nc, in_maps, core_ids=list(range(8)), trace=False
        )
    LAST_RESULTS = res

    out = np.empty((B, S, D), np.float32)
    for core in range(8):
        b, j = core // 2, core % 2
        op = res.results[core]["out"].astype(np.float32)
        for p in range(8):
            t = 2 * p + j
            out[b, t * P : (t + 1) * P, :] = op[p * P : (p + 1) * P, :]
    return out

